# revision 23
# baseline (speedup 1.0000x reference)
"""Trainium2 Bass kernel for BinaryMaskEdgeSmoothing.

Reference computation (per image, SAME-padded 3x3 convs):
    e  = conv3x3(x, lap)
    em = sigmoid(|e| * 3)
    b  = conv3x3(x, gauss)
    smoothed = x*(1-em) + b*em
    out = (smoothed > 0.5).astype(f32)

Device decomposition (per NeuronCore, data-parallel over B*C=64 images,
8 images per core):

  * Images are processed in row-tiles of 128 rows (partition dim = image
    rows, free dim = the 1024 columns).  The vertical direction of each
    3x3 conv is computed on the TensorEngine as a banded-matrix matmul
    (lhsT[p, m] = k[dy, dx] at p = m+dy-1); the horizontal taps are
    free-dim shifts of the moving operand, accumulated into the same
    PSUM bank over the three dx matmuls.  So each conv costs 3 matmuls
    per 512-column PSUM bank and the full 3x3 conv lands in PSUM with
    zero vector-engine work.
  * Rather than computing gauss and then b-x, the PE directly computes
    d = conv(x, gauss - delta) = b - x  (delta = unit impulse), which is
    exact and saves elementwise work.
  * The moving operand is the *negated* mask in bf16 (exact for a 0/1
    mask).  conv(-x) = -conv(x); |e| is unaffected and the sign of d is
    absorbed into the final fused compare:
        out = 1  iff  em*(b-x) + x > 0.5  iff  t_neg + 0.5 < x
    where t_neg = em * conv(-x, gauss-delta).  That final compare is a
    single fused GpSimd scalar_tensor_tensor op.
  * Row tiles overlap by 2 rows (stride 126) so every output row has its
    vertical halo inside one tile; rows 1009..1023 of all 8 images are
    batched into one final tile with block-diagonal band matrices.

Engine budget per [128,1024] unit: PE 12 matmuls, ACT abs+sigmoid,
DVE cast+mul, GPSIMD fused compare, DMA 0.5MiB in + ~0.5MiB out.
Everything lands near the per-core HBM roofline (~64MiB @ ~360GB/s).
"""

import numpy as np
import ml_dtypes

import concourse.bass as bass
import concourse.bacc as bacc
import concourse.mybir as mybir
import concourse.tile as tile
from concourse.bass_utils import run_bass_kernel_spmd

Af = mybir.ActivationFunctionType
Op = mybir.AluOpType
F32 = mybir.dt.float32
BF16 = mybir.dt.bfloat16

N_CORES = 8
B_PER_CORE = 8
H = 1024
W = 1024

# Row tiling: main units at i0 = 126*k, each 128 input rows.
#   k == 0 : output rows 0..126   (partitions 0..126)
#   k >= 1 : output rows i0+1..i0+126 (partitions 1..126)
# Tail: remaining rows handled by one batched unit across all images.


def _tiling(h):
    n_main = (h - 128) // 126 + 1
    covered_max = 126 * (n_main - 1) + 126
    r = h - 1 - covered_max  # rows left for the tail unit
    return n_main, r


def build_weights(lap_kernel, gauss_kernel, b_imgs=B_PER_CORE, h=H):
    """Host-side: 12 [128,128] banded lhsT matrices -> [128, 12*128] bf16.

    idx = kern*3 + dx        : mid-tile band (shared by all main units)
    idx = 6 + kern*3 + dx    : tail-unit block-diagonal band
    """
    lap = np.asarray(lap_kernel, dtype=np.float64).reshape(3, 3)
    gau = np.asarray(gauss_kernel, dtype=np.float64).reshape(3, 3)
    gmd = gau.copy()
    gmd[1, 1] -= 1.0  # gauss - delta  ->  conv = b - x
    kerns = [lap, gmd]

    _, r = _tiling(h)
    s = r + 1  # tail block height (input rows per image in tail unit)

    mats = np.zeros((12, 128, 128), dtype=np.float64)
    for ki, k3 in enumerate(kerns):
        for dx in range(3):
            m = mats[ki * 3 + dx]
            for out_m in range(128):
                for dy in range(3):
                    p = out_m + dy - 1
                    if 0 <= p < 128:
                        m[p, out_m] = k3[dy, dx]
            mt = mats[6 + ki * 3 + dx]
            if r > 0:
                for blk in range(b_imgs):
                    base = blk * s
                    for out_m in range(s):
                        for dy in range(3):
                            p = out_m + dy - 1
                            if 0 <= p < s:
                                mt[base + p, base + out_m] = k3[dy, dx]
    # stack as [p, idx, m] so one DMA drops it straight into SBUF
    w = np.transpose(mats, (1, 0, 2)).reshape(128, 12 * 128)
    return np.ascontiguousarray(w.astype(np.float32)).astype(ml_dtypes.bfloat16)


def build_nc(b_imgs=B_PER_CORE, h=H, w=W, rep=1, rep_loop=0, ablate=None, store_engine="scalar"):
    n_main, r = _tiling(h)
    s = r + 1
    if r > 0:
        assert b_imgs * s <= 128, (b_imgs, s)
    # W chunking into PSUM banks (<=512 f32 per matmul free dim)
    chunks = []
    c0 = 0
    while c0 < w:
        chunks.append((c0, min(c0 + 512, w)))
        c0 += 512

    nc = bacc.Bacc()
    x_d = nc.declare_dram_parameter("x", [b_imgs, h, w], F32, isOutput=False)
    w_d = nc.declare_dram_parameter("wts", [128, 12 * 128], BF16, isOutput=False)
    o_d = nc.declare_dram_parameter("out", [b_imgs, h, w], F32, isOutput=True)

    with tile.TileContext(nc) as tc:
        with (
            tc.tile_pool(name="const", bufs=1) as cpool,
            tc.tile_pool(name="xin", bufs=2) as xpool,
            tc.tile_pool(name="work", bufs=3) as wpool,
            tc.tile_pool(name="oput", bufs=2) as opool,
            tc.tile_pool(name="psum", bufs=2, space="PSUM") as ppool,
        ):
            st_eng = nc.scalar if store_engine == "scalar" else nc.sync
            wsb = cpool.tile([128, 12 * 128], BF16)
            nc.sync.dma_start(wsb[:], w_d[:])
            neghalf = cpool.tile([128, w], F32)
            nc.vector.memset(neghalf[:], -0.5)

            def lhsT(variant, kern, dx, kpart=128):
                idx = variant * 6 + kern * 3 + dx
                return wsb[0:kpart, idx * 128:(idx + 1) * 128]

            def lhsT_m(variant, kern, dx, kpart, mpart):
                idx = variant * 6 + kern * 3 + dx
                return wsb[0:kpart, idx * 128:idx * 128 + mpart]

            def conv_unit(xf, xneg, variant, o_ap, kpart=128, mpart=128):
                """emit 12 matmuls + pointwise chain; writes o_ap."""
                e_ps = ppool.tile([mpart, w], F32, tag="e")
                d_ps = ppool.tile([mpart, w], F32, tag="d")
                for kern, ps in ((0, e_ps), (1, d_ps)):
                    for (a, b) in chunks:
                        # center tap (dx=1) covers the whole bank: start=True
                        nc.tensor.matmul(
                            ps[:, a:b], lhsT_m(variant, kern, 1, kpart, mpart),
                            xneg[:, a:b], start=True, stop=False)
                        # left neighbor (dx=0): out[:, j] += k*x[:, j-1]
                        la = max(a, 1)
                        nc.tensor.matmul(
                            ps[:, la:b], lhsT_m(variant, kern, 0, kpart, mpart),
                            xneg[:, la - 1:b - 1], start=False, stop=False)
                        # right neighbor (dx=2): out[:, j] += k*x[:, j+1]
                        rb = min(b, w - 1)
                        nc.tensor.matmul(
                            ps[:, a:rb], lhsT_m(variant, kern, 2, kpart, mpart),
                            xneg[:, a + 1:rb + 1], start=False, stop=True)
                a_sb = wpool.tile([mpart, w], F32, tag="a")
                nc.scalar.activation(a_sb[:], e_ps[:], Af.Abs)
                if ablate == "conv":
                    nc.scalar.activation(o_ap, d_ps[:], Af.Abs)
                    return
                em_sb = wpool.tile([mpart, w], F32, tag="em")
                nc.scalar.activation(em_sb[:], a_sb[:], Af.Sigmoid, scale=3.0)
                t_sb = wpool.tile([mpart, w], F32, tag="t")
                nc.vector.tensor_tensor(t_sb[:], em_sb[:], d_ps[:], Op.mult)
                # out = ((t_neg + 0.5) < x)  <=>  smoothed > 0.5  -> 1.0/0.0
                # fused on DVE; GPSIMD is unused (its TT ucode is ~2x slower
                # than DVE and blocks on the shared SBUF port)
                nc.vector.scalar_tensor_tensor(
                    o_ap, t_sb[:], 0.5, xf, Op.add, Op.is_lt)

            import contextlib

            def body_ctx():
                if rep_loop:
                    return tc.For_i(0, rep_loop, 1)
                return contextlib.nullcontext()

            # LG row-tiles per load transfer, SG per store transfer
            # (loads on the SP HWDGE ring, stores separate so store waits
            # never stall the load FIFO)
            LG = 8 if n_main % 8 == 0 else (4 if n_main % 4 == 0 else 1)
            SG = min(4, LG)

            with body_ctx():
              for _ in range(rep):
                for b in range(b_imgs):
                    for l0 in range(0, n_main, LG):
                        lg = min(LG, n_main - l0)
                        xt = xpool.tile([128, lg, w], F32, tag="xf")
                        nc.sync.dma_start(
                            xt[:],
                            bass.AP(x_d, (b * h + 126 * l0) * w,
                                    [[w, 128], [126 * w, lg], [1, w]]))
                        if ablate != "dma":
                            xneg = xpool.tile([128, lg, w], BF16, tag="xn")
                            nc.vector.tensor_scalar(
                                xneg[:], xt[:], -1.0, None, Op.mult)
                        for k0 in range(l0, l0 + lg, SG):
                            gu = min(SG, l0 + lg - k0)
                            if ablate == "dma":
                                o_grp = xt
                                osl = slice(k0 - l0, k0 - l0 + gu)
                            else:
                                o_grp = opool.tile(
                                    [128, gu, w], F32, tag="o")
                                osl = slice(0, gu)
                                for j in range(gu):
                                    u = k0 - l0 + j
                                    conv_unit(xt[:, u, :], xneg[:, u, :],
                                              0, o_grp[:, j, :])
                            # batched store: rows 126*k0+1 ..
                            st_eng.dma_start(
                                bass.AP(o_d, (b * h + 126 * k0 + 1) * w,
                                        [[w, 126], [126 * w, gu], [1, w]]),
                                o_grp[1:127, osl, :])
                            if k0 == 0:
                                st_eng.dma_start(
                                    o_d[b, 0:1, :], o_grp[0:1, osl.start, :])

                if r > 0:
                    kpart = b_imgs * s
                    hc = h - s
                    xft = xpool.tile([kpart, w], F32, tag="xft")
                    nc.sync.dma_start(xft[:], x_d[:, hc:h, :])
                    xnt = xpool.tile([kpart, w], BF16, tag="xnt")
                    nc.vector.tensor_scalar(
                        xnt[:], xft[:], -1.0, None, Op.mult)
                    o_t = opool.tile([kpart, w], F32, tag="ot")
                    conv_unit(xft[:], xnt[:], 1, o_t[:], kpart, kpart)
                    for b in range(b_imgs):
                        st_eng.dma_start(
                            o_d[b, hc + 1:h, :], o_t[b * s + 1:(b + 1) * s, :])

    return nc


_NC_CACHE = {}


def _get_nc(key=(B_PER_CORE, H, W)):
    if key not in _NC_CACHE:
        nc = build_nc(*key)
        nc.finalize()
        _NC_CACHE[key] = nc
    return _NC_CACHE[key]


def kernel(mask, lap_kernel, gauss_kernel):
    mask = np.ascontiguousarray(np.asarray(mask, dtype=np.float32))
    bb, cc, h, w = mask.shape
    assert (h, w) == (H, W) and bb * cc == N_CORES * B_PER_CORE
    x_all = mask.reshape(N_CORES * B_PER_CORE, h, w)
    wts = build_weights(lap_kernel, gauss_kernel)

    nc = _get_nc()
    in_maps = [
        {"x": np.ascontiguousarray(x_all[c * B_PER_CORE:(c + 1) * B_PER_CORE]),
         "wts": wts}
        for c in range(N_CORES)
    ]
    res = run_bass_kernel_spmd(nc, in_maps, list(range(N_CORES)))
    out = np.stack([res.results[c]["out"] for c in range(N_CORES)])
    return out.reshape(bb, cc, h, w).astype(np.float32)


# revision 24
# speedup vs baseline: 1.0730x; 1.0730x over previous
"""Trainium2 Bass kernel for BinaryMaskEdgeSmoothing.

Reference computation (per image, SAME-padded 3x3 convs):
    e  = conv3x3(x, lap)
    em = sigmoid(|e| * 3)
    b  = conv3x3(x, gauss)
    smoothed = x*(1-em) + b*em
    out = (smoothed > 0.5).astype(f32)

Device decomposition (per NeuronCore, data-parallel over B*C=64 images,
8 images per core):

  * Images are processed in row-tiles of 128 rows (partition dim = image
    rows, free dim = the 1024 columns).  The vertical direction of each
    3x3 conv is computed on the TensorEngine as a banded-matrix matmul
    (lhsT[p, m] = k[dy, dx] at p = m+dy-1); the horizontal taps are
    free-dim shifts of the moving operand, accumulated into the same
    PSUM bank over the three dx matmuls.  So each conv costs 3 matmuls
    per 512-column PSUM bank and the full 3x3 conv lands in PSUM with
    zero vector-engine work.
  * Rather than computing gauss and then b-x, the PE directly computes
    d = conv(x, gauss - delta) = b - x  (delta = unit impulse), which is
    exact and saves elementwise work.
  * The moving operand is the *negated* mask in bf16 (exact for a 0/1
    mask).  conv(-x) = -conv(x); |e| is unaffected and the sign of d is
    absorbed into the final fused compare:
        out = 1  iff  em*(b-x) + x > 0.5  iff  t_neg + 0.5 < x
    where t_neg = em * conv(-x, gauss-delta).  That final compare is a
    single fused GpSimd scalar_tensor_tensor op.
  * Row tiles overlap by 2 rows (stride 126) so every output row has its
    vertical halo inside one tile; rows 1009..1023 of all 8 images are
    batched into one final tile with block-diagonal band matrices.

Engine budget per [128,1024] unit: PE 12 matmuls, ACT abs+sigmoid,
DVE cast+mul, GPSIMD fused compare, DMA 0.5MiB in + ~0.5MiB out.
Everything lands near the per-core HBM roofline (~64MiB @ ~360GB/s).
"""

import numpy as np
import ml_dtypes

import concourse.bass as bass
import concourse.bacc as bacc
import concourse.mybir as mybir
import concourse.tile as tile
from concourse.bass_utils import run_bass_kernel_spmd

Af = mybir.ActivationFunctionType
Op = mybir.AluOpType
F32 = mybir.dt.float32
BF16 = mybir.dt.bfloat16

N_CORES = 8
B_PER_CORE = 8
H = 1024
W = 1024

# Row tiling: main units at i0 = 126*k, each 128 input rows.
#   k == 0 : output rows 0..126   (partitions 0..126)
#   k >= 1 : output rows i0+1..i0+126 (partitions 1..126)
# Tail: remaining rows handled by one batched unit across all images.


def _tiling(h):
    n_main = (h - 128) // 126 + 1
    covered_max = 126 * (n_main - 1) + 126
    r = h - 1 - covered_max  # rows left for the tail unit
    return n_main, r


def build_weights(lap_kernel, gauss_kernel, b_imgs=B_PER_CORE, h=H):
    """Host-side: 12 [128,128] banded lhsT matrices -> [128, 12*128] bf16.

    idx = kern*3 + dx        : mid-tile band (shared by all main units)
    idx = 6 + kern*3 + dx    : tail-unit block-diagonal band
    """
    lap = np.asarray(lap_kernel, dtype=np.float64).reshape(3, 3)
    gau = np.asarray(gauss_kernel, dtype=np.float64).reshape(3, 3)
    gmd = gau.copy()
    gmd[1, 1] -= 1.0  # gauss - delta  ->  conv = b - x
    kerns = [lap, gmd]

    _, r = _tiling(h)
    s = r + 1  # tail block height (input rows per image in tail unit)

    mats = np.zeros((12, 128, 128), dtype=np.float64)
    for ki, k3 in enumerate(kerns):
        for dx in range(3):
            m = mats[ki * 3 + dx]
            for out_m in range(128):
                for dy in range(3):
                    p = out_m + dy - 1
                    if 0 <= p < 128:
                        m[p, out_m] = k3[dy, dx]
            mt = mats[6 + ki * 3 + dx]
            if r > 0:
                for blk in range(b_imgs):
                    base = blk * s
                    for out_m in range(s):
                        for dy in range(3):
                            p = out_m + dy - 1
                            if 0 <= p < s:
                                mt[base + p, base + out_m] = k3[dy, dx]
    # stack as [p, idx, m] so one DMA drops it straight into SBUF
    w = np.transpose(mats, (1, 0, 2)).reshape(128, 12 * 128)
    return np.ascontiguousarray(w.astype(np.float32)).astype(ml_dtypes.bfloat16)


def build_nc(b_imgs=B_PER_CORE, h=H, w=W, rep=1, rep_loop=0, ablate=None, store_engine="scalar"):
    n_main, r = _tiling(h)
    s = r + 1
    if r > 0:
        assert b_imgs * s <= 128, (b_imgs, s)
    # W chunking into PSUM banks (<=512 f32 per matmul free dim)
    chunks = []
    c0 = 0
    while c0 < w:
        chunks.append((c0, min(c0 + 512, w)))
        c0 += 512

    nc = bacc.Bacc()
    x_d = nc.declare_dram_parameter("x", [b_imgs, h, w], F32, isOutput=False)
    w_d = nc.declare_dram_parameter("wts", [128, 12 * 128], BF16, isOutput=False)
    o_d = nc.declare_dram_parameter("out", [b_imgs, h, w], F32, isOutput=True)

    with tile.TileContext(nc) as tc:
        with (
            tc.tile_pool(name="const", bufs=1) as cpool,
            tc.tile_pool(name="xin", bufs=3) as xpool,
            tc.tile_pool(name="work", bufs=3) as wpool,
            tc.tile_pool(name="oput", bufs=3) as opool,
            tc.tile_pool(name="psum", bufs=2, space="PSUM") as ppool,
        ):
            st_eng = nc.scalar if store_engine == "scalar" else nc.sync
            wsb = cpool.tile([128, 12 * 128], BF16)
            nc.sync.dma_start(wsb[:], w_d[:])
            neghalf = cpool.tile([128, w], F32)
            nc.vector.memset(neghalf[:], -0.5)

            def lhsT(variant, kern, dx, kpart=128):
                idx = variant * 6 + kern * 3 + dx
                return wsb[0:kpart, idx * 128:(idx + 1) * 128]

            def lhsT_m(variant, kern, dx, kpart, mpart):
                idx = variant * 6 + kern * 3 + dx
                return wsb[0:kpart, idx * 128:idx * 128 + mpart]

            def conv_unit(xf, xneg, variant, o_ap, kpart=128, mpart=128):
                """emit 12 matmuls + pointwise chain; writes o_ap."""
                e_ps = ppool.tile([mpart, w], F32, tag="e")
                d_ps = ppool.tile([mpart, w], F32, tag="d")
                for kern, ps in ((0, e_ps), (1, d_ps)):
                    for (a, b) in chunks:
                        # center tap (dx=1) covers the whole bank: start=True
                        nc.tensor.matmul(
                            ps[:, a:b], lhsT_m(variant, kern, 1, kpart, mpart),
                            xneg[:, a:b], start=True, stop=False)
                        # left neighbor (dx=0): out[:, j] += k*x[:, j-1]
                        la = max(a, 1)
                        nc.tensor.matmul(
                            ps[:, la:b], lhsT_m(variant, kern, 0, kpart, mpart),
                            xneg[:, la - 1:b - 1], start=False, stop=False)
                        # right neighbor (dx=2): out[:, j] += k*x[:, j+1]
                        rb = min(b, w - 1)
                        nc.tensor.matmul(
                            ps[:, a:rb], lhsT_m(variant, kern, 2, kpart, mpart),
                            xneg[:, a + 1:rb + 1], start=False, stop=True)
                a_sb = wpool.tile([mpart, w], F32, tag="a")
                nc.scalar.activation(a_sb[:], e_ps[:], Af.Abs)
                if ablate == "conv":
                    nc.scalar.activation(o_ap, d_ps[:], Af.Abs)
                    return
                em_sb = wpool.tile([mpart, w], F32, tag="em")
                nc.scalar.activation(em_sb[:], a_sb[:], Af.Sigmoid, scale=3.0)
                t_sb = wpool.tile([mpart, w], F32, tag="t")
                nc.vector.tensor_tensor(t_sb[:], em_sb[:], d_ps[:], Op.mult)
                # out = ((t_neg + 0.5) < x)  <=>  smoothed > 0.5  -> 1.0/0.0
                # fused on DVE; GPSIMD is unused (its TT ucode is ~2x slower
                # than DVE and blocks on the shared SBUF port)
                nc.vector.scalar_tensor_tensor(
                    o_ap, t_sb[:], 0.5, xf, Op.add, Op.is_lt)

            import contextlib

            def body_ctx():
                if rep_loop:
                    return tc.For_i(0, rep_loop, 1)
                return contextlib.nullcontext()

            # LG row-tiles per load transfer, SG per store transfer
            # (loads on the SP HWDGE ring, stores separate so store waits
            # never stall the load FIFO)
            LG = 4 if n_main % 4 == 0 else 1
            SG = min(4, LG)

            with body_ctx():
              for _ in range(rep):
                for b in range(b_imgs):
                    for l0 in range(0, n_main, LG):
                        lg = min(LG, n_main - l0)
                        xt = xpool.tile([128, lg, w], F32, tag="xf")
                        nc.sync.dma_start(
                            xt[:],
                            bass.AP(x_d, (b * h + 126 * l0) * w,
                                    [[w, 128], [126 * w, lg], [1, w]]))
                        if ablate != "dma":
                            xneg = xpool.tile([128, lg, w], BF16, tag="xn")
                            nc.vector.tensor_scalar(
                                xneg[:], xt[:], -1.0, None, Op.mult)
                        for k0 in range(l0, l0 + lg, SG):
                            gu = min(SG, l0 + lg - k0)
                            if ablate == "dma":
                                o_grp = xt
                                osl = slice(k0 - l0, k0 - l0 + gu)
                            else:
                                o_grp = opool.tile(
                                    [128, gu, w], F32, tag="o")
                                osl = slice(0, gu)
                                for j in range(gu):
                                    u = k0 - l0 + j
                                    conv_unit(xt[:, u, :], xneg[:, u, :],
                                              0, o_grp[:, j, :])
                            # batched store: rows 126*k0+1 ..
                            st_eng.dma_start(
                                bass.AP(o_d, (b * h + 126 * k0 + 1) * w,
                                        [[w, 126], [126 * w, gu], [1, w]]),
                                o_grp[1:127, osl, :])
                            if k0 == 0:
                                st_eng.dma_start(
                                    o_d[b, 0:1, :], o_grp[0:1, osl.start, :])

                if r > 0:
                    kpart = b_imgs * s
                    hc = h - s
                    xft = xpool.tile([kpart, w], F32, tag="xft")
                    nc.sync.dma_start(xft[:], x_d[:, hc:h, :])
                    xnt = xpool.tile([kpart, w], BF16, tag="xnt")
                    nc.vector.tensor_scalar(
                        xnt[:], xft[:], -1.0, None, Op.mult)
                    o_t = opool.tile([kpart, w], F32, tag="ot")
                    conv_unit(xft[:], xnt[:], 1, o_t[:], kpart, kpart)
                    for b in range(b_imgs):
                        st_eng.dma_start(
                            o_d[b, hc + 1:h, :], o_t[b * s + 1:(b + 1) * s, :])

    return nc


_NC_CACHE = {}


def _get_nc(key=(B_PER_CORE, H, W)):
    if key not in _NC_CACHE:
        nc = build_nc(*key)
        nc.finalize()
        _NC_CACHE[key] = nc
    return _NC_CACHE[key]


def kernel(mask, lap_kernel, gauss_kernel):
    mask = np.ascontiguousarray(np.asarray(mask, dtype=np.float32))
    bb, cc, h, w = mask.shape
    assert (h, w) == (H, W) and bb * cc == N_CORES * B_PER_CORE
    x_all = mask.reshape(N_CORES * B_PER_CORE, h, w)
    wts = build_weights(lap_kernel, gauss_kernel)

    nc = _get_nc()
    in_maps = [
        {"x": np.ascontiguousarray(x_all[c * B_PER_CORE:(c + 1) * B_PER_CORE]),
         "wts": wts}
        for c in range(N_CORES)
    ]
    res = run_bass_kernel_spmd(nc, in_maps, list(range(N_CORES)))
    out = np.stack([res.results[c]["out"] for c in range(N_CORES)])
    return out.reshape(bb, cc, h, w).astype(np.float32)
